# revision 20
# baseline (speedup 1.0000x reference)
"""Trainium2 Bass kernel for nn_Loss_17695265260053 (retrieval_knn).

Computes, for B=16 batches of N=2048 3-D points:
  sym[b]  = mean_n min_m ||pred[b,n] - targ[b,m]||      (Chamfer / ADD-S)
  asym[b] = mean_n ||pred[b,n] - targ[b,n]||            (ADD)
  loss    = mean_b (flag[b]*sym[b] + (1-flag[b])*asym[b])

Sharding: data-parallel over batch, 2 batches per core on 8 cores; each
core emits one partial sum, the host sums partials and divides by B.

Per-core algorithm (per batch):
  d2'(n,m) = |t_m|^2 - 2 p_n.t_m          (|p_n|^2 is added after the min)
  fp32 operands are split error-free into fp16 hi+lo halves and the three
  significant products are contracted in a SINGLE K=12 fp16 matmul
  (lhsT = [X_hi; X_hi; X_lo], rhs = [A_hi; A_lo; A_hi], X = (p, 1),
  A = (-2t, |t|^2)); the dropped lo*lo term is ~1e-7 relative. fp16
  streams at 4x the fp32 matmul rate on the PE.
  Per 128-row pred tile the 2048 d2' values land in one [128,2048] PSUM
  tile (4 banks, double-buffered across the 8); VectorE min-reduces it in
  a single fused tensor_scalar(op1=min, accum_out) pass from PSUM.
  Epilogue: + |p|^2, clamp EPS, Sqrt, sum-reduce; a ones-matmul reduces
  across partitions; the sym_flag blend happens on [1,x] lanes.
"""

import sys

for _p in ("/opt/trn_rl_repo", "/opt/pypackages"):
    if _p not in sys.path:
        sys.path.insert(0, _p)

import numpy as np

import concourse.bass as bass
import concourse.tile as tile
from concourse import bacc, mybir

N_CORES = 8
B, N, D = 16, 2048, 3
BPC = B // N_CORES          # batches per core
NT = N // 128               # 16 pred tiles of 128 points
NW = 2048                   # full-width PSUM tile per pred tile
F32 = mybir.dt.float32
F16 = mybir.dt.float16
EPS = 1e-12
Alu = mybir.AluOpType
Act = mybir.ActivationFunctionType


def build_loss_body(nc, tc, predt_d, prednat_d, targnat_d, flag_d, out_d, stage_d):
    """Emit the per-core program.
    predt_d:   [BPC, 12, N] f16 - rows [X_hi; X_hi; X_lo], X = (p, 1) transposed
    prednat_d: [BPC, 128, 48] f32 - tiled natural pred ([q, 3t+d] = pt 128t+q)
    targnat_d: [BPC, 128, 48] f32 - tiled natural target
    flag_d: [1, BPC]; out_d: [1, 1]; stage_d: [BPC, 128, 128] f16."""
    with (
        tc.tile_pool(name="io", bufs=2) as io,
        tc.tile_pool(name="pre", bufs=2) as pre,
        tc.tile_pool(name="rhs", bufs=2) as rhsp,
        tc.tile_pool(name="work", bufs=3) as work,
        tc.tile_pool(name="acc", bufs=1) as accp,
        tc.tile_pool(name="psum", bufs=2, space="PSUM") as psum,
    ):
        # per-core accumulators / constants
        SSUM = accp.tile([128, 2 * BPC], F32)   # cols: sym0, asym0, sym1, asym1
        ONES = accp.tile([128, 1], F32)
        nc.vector.memset(ONES[:], 1.0 / N)      # folds the 1/N mean into the reduce
        FL = accp.tile([1, BPC], F32)
        nc.sync.dma_start(FL[:], flag_d[:])
        TRA = accp.tile([128, NW], F32)         # reduce elementwise dump

        for b in range(BPC):
            # ---- loads ------------------------------------------------
            P4 = io.tile([128, NT * 3], F32, tag="P4")
            nc.sync.dma_start(P4[:], prednat_d[b])
            T4 = io.tile([128, NT * 3], F32, tag="T4")
            nc.sync.dma_start(T4[:], targnat_d[b])
            LT = rhsp.tile([12, N], F16, tag="LT")
            nc.sync.dma_start(LT[:], predt_d[b])

            # ---- target prep: rhs rows (-2*t coords, |t|^2), fp32 ------
            T4SQ = pre.tile([128, NT * 3], F32, tag="t4sq")
            nc.scalar.activation(T4SQ[:], T4[:], Act.Square)
            tv = T4SQ.rearrange("q (t d) -> q t d", d=3)

            # PREP[q, c*16 + t]: c 0..2 = -2*t coords, c 3 = |t|^2
            PREP = pre.tile([128, 64], F32, tag="prep")
            pv = PREP.rearrange("q (c t) -> q c t", c=4)
            T4dt = T4.rearrange("q (t d) -> q t d", d=3).transpose([0, 2, 1])
            nc.vector.tensor_scalar_mul(pv[:, 0:3, :], T4dt[:, :, :], -2.0)
            t2tmp = pre.tile([128, NT], F32, tag="t2t")
            nc.vector.tensor_add(t2tmp[:], tv[:, :, 0], tv[:, :, 1])
            nc.vector.tensor_add(pv[:, 3, :], t2tmp[:], tv[:, :, 2])

            # error-free fp16 hi/lo split of PREP
            PREPH = pre.tile([128, 64], F16, tag="preph")
            nc.vector.tensor_copy(PREPH[:], PREP[:])
            PUP = pre.tile([128, 64], F32, tag="pup")
            nc.vector.tensor_copy(PUP[:], PREPH[:])
            PLF = pre.tile([128, 64], F32, tag="plf")
            nc.vector.tensor_sub(PLF[:], PREP[:], PUP[:])
            PREPL = pre.tile([128, 64], F16, tag="prepl")
            nc.vector.tensor_copy(PREPL[:], PLF[:])

            # stage to DRAM transposed ([x, q], q contiguous); gather back as
            # the K=12 rhs: RT[4g+c, t*128+q] = blk(g)[q, c*16+t], blk=(hi,lo,hi)
            nc.gpsimd.dma_start(stage_d[b, 0:64].rearrange("x q -> q x"), PREPH[:])
            nc.gpsimd.dma_start(stage_d[b, 64:128].rearrange("x q -> q x"), PREPL[:])
            RT = rhsp.tile([12, N], F16, tag="RT")
            hi_src = stage_d[b, 0:64].rearrange("(c t) q -> c t q", c=4)
            lo_src = stage_d[b, 64:128].rearrange("(c t) q -> c t q", c=4)
            rtv = RT.rearrange("c (t q) -> c t q", q=128)
            nc.gpsimd.dma_start(rtv[0:4], hi_src)
            nc.gpsimd.dma_start(rtv[4:8], lo_src)
            nc.gpsimd.dma_start(rtv[8:12], hi_src)

            # ---- pred prep: |p|^2 and the asym (ADD) branch ------------
            P4SQ = pre.tile([128, NT * 3], F32, tag="p4sq")
            nc.scalar.activation(P4SQ[:], P4[:], Act.Square)
            pv2 = P4SQ.rearrange("q (t d) -> q t d", d=3)
            p2t = pre.tile([128, NT], F32, tag="p2t")
            nc.vector.tensor_add(p2t[:], pv2[:, :, 0], pv2[:, :, 1])
            nc.vector.tensor_add(p2t[:], p2t[:], pv2[:, :, 2])

            ADIF = pre.tile([128, NT * 3], F32, tag="adif")
            nc.vector.tensor_sub(ADIF[:], P4[:], T4[:])
            ASQ = pre.tile([128, NT * 3], F32, tag="asq")
            nc.scalar.activation(ASQ[:], ADIF[:], Act.Square)
            av = ASQ.rearrange("q (t d) -> q t d", d=3)
            AD2 = pre.tile([128, NT], F32, tag="ad2")
            nc.vector.tensor_add(AD2[:], av[:, :, 0], av[:, :, 1])
            nc.vector.tensor_add(AD2[:], AD2[:], av[:, :, 2])
            ASQR = pre.tile([128, NT], F32, tag="asqr")
            nc.scalar.activation(ASQR[:], AD2[:], Act.Sqrt)
            nc.vector.reduce_sum(
                SSUM[:, 2 * b + 1 : 2 * b + 2], ASQR[:], axis=mybir.AxisListType.X
            )

            # ---- main loop: K=12 fp16 matmuls + fused min-reduce -------
            MINS = pre.tile([128, NT], F32, tag="mins")
            for a in range(NT):
                lhs = LT[:, 128 * a : 128 * (a + 1)]
                ps = psum.tile([128, NW], F32, tag="ps")
                if a == 0:
                    # 1-col "toucher" ladder: spread the batch-boundary waits
                    # (psum WAR/WAW, LT DMA, RT gathers) over cheap matmuls so
                    # no single LDWEIGHTS exceeds its sync-wait budget.
                    nc.tensor.matmul(
                        ps[0:1, 0:1], ONES[:], ONES[:], start=True, stop=True
                    )
                    nc.tensor.matmul(
                        ps[0:1, 1:2], LT[:, 0:1], LT[:, 0:1], start=True, stop=True
                    )
                    nc.tensor.matmul(
                        ps[0:1, 2:3], RT[:, 0:1], RT[:, 0:1], start=True, stop=True
                    )
                for c in range(4):
                    nc.tensor.matmul(
                        ps[:, 512 * c : 512 * (c + 1)],
                        lhs,
                        RT[:, 512 * c : 512 * (c + 1)],
                        start=True,
                        stop=True,
                    )
                nc.vector.tensor_scalar(
                    TRA[:], ps[:], 0.0, None,
                    op0=Alu.add, op1=Alu.min, accum_out=MINS[:, a : a + 1],
                )

            # ---- epilogue: + |p|^2, clamp, sqrt ------------------------
            D2M = pre.tile([128, NT], F32, tag="d2m")
            nc.vector.tensor_add(D2M[:], p2t[:], MINS[:])
            nc.vector.tensor_scalar_max(D2M[:], D2M[:], EPS)
            DSQ = pre.tile([128, NT], F32, tag="dsq")
            nc.scalar.activation(DSQ[:], D2M[:], Act.Sqrt)
            nc.vector.reduce_sum(
                SSUM[:, 2 * b : 2 * b + 1], DSQ[:], axis=mybir.AxisListType.X
            )

        # ---- final: partition reduce + flag blend ----------------------
        FPS = psum.tile([1, 2 * BPC], F32, tag="ps")
        nc.tensor.matmul(FPS[:], ONES[:], SSUM[:], start=True, stop=True)
        FSB = accp.tile([1, 2 * BPC], F32)
        nc.vector.tensor_copy(FSB[:], FPS[:])
        fv = FSB.rearrange("p (b k) -> p b k", k=2)  # k: 0 = sym, 1 = asym
        T0 = accp.tile([1, BPC], F32)
        nc.vector.tensor_sub(T0[:], fv[:, :, 0], fv[:, :, 1])
        nc.vector.tensor_mul(T0[:], T0[:], FL[:])
        nc.vector.tensor_add(T0[:], T0[:], fv[:, :, 1])
        OUT = accp.tile([1, 1], F32)
        nc.vector.reduce_sum(OUT[:], T0[:], axis=mybir.AxisListType.X)
        nc.sync.dma_start(out_d[:], OUT[:])


def build_core_program():
    """Build the single-core Bass program (same program runs SPMD on all 8)."""
    nc = bacc.Bacc("TRN2", target_bir_lowering=False, debug=False)
    predt_d = nc.dram_tensor("predt", [BPC, 12, N], F16, kind="ExternalInput")
    prednat_d = nc.dram_tensor("prednat", [BPC, 128, NT * 3], F32, kind="ExternalInput")
    targnat_d = nc.dram_tensor("targnat", [BPC, 128, NT * 3], F32, kind="ExternalInput")
    flag_d = nc.dram_tensor("flag", [1, BPC], F32, kind="ExternalInput")
    out_d = nc.dram_tensor("out", [1, 1], F32, kind="ExternalOutput")
    stage_d = nc.dram_tensor("stage", [BPC, 128, 128], F16)
    with tile.TileContext(nc) as tc:
        build_loss_body(nc, tc, predt_d.ap(), prednat_d.ap(), targnat_d.ap(),
                        flag_d.ap(), out_d.ap(), stage_d.ap())
    nc.compile()
    return nc


def host_inputs(pred_points, targ_points, sym_flag):
    """Host-side input formatting (shard + layout/precision split only)."""
    pred = np.asarray(pred_points, dtype=np.float32)
    targ = np.asarray(targ_points, dtype=np.float32)
    pred4 = np.concatenate([pred, np.ones((B, N, 1), np.float32)], axis=-1)
    ph = pred4.astype(np.float16)
    pl = (pred4 - ph.astype(np.float32)).astype(np.float16)
    # [B, 12, N]: rows [X_hi; X_hi; X_lo]
    predt = np.concatenate(
        [ph.transpose(0, 2, 1), ph.transpose(0, 2, 1), pl.transpose(0, 2, 1)], axis=1
    )
    tiled = lambda x: np.ascontiguousarray(
        x.reshape(B, NT, 128, 3).transpose(0, 2, 1, 3).reshape(B, 128, NT * 3)
    )
    prednat, targnat = tiled(pred), tiled(targ)
    flags = np.asarray(sym_flag, dtype=np.float32)
    return predt, prednat, targnat, flags


def make_in_maps(pred_points, targ_points, sym_flag):
    predt, prednat, targnat, flags = host_inputs(pred_points, targ_points, sym_flag)
    in_maps = []
    for c in range(N_CORES):
        sl = slice(c * BPC, (c + 1) * BPC)
        in_maps.append(
            {
                "predt": np.ascontiguousarray(predt[sl]),
                "prednat": np.ascontiguousarray(prednat[sl]),
                "targnat": np.ascontiguousarray(targnat[sl]),
                "flag": np.ascontiguousarray(flags[sl].reshape(1, BPC)),
            }
        )
    return in_maps


_NC_CACHE = None


def _get_nc():
    global _NC_CACHE
    if _NC_CACHE is None:
        _NC_CACHE = build_core_program()
    return _NC_CACHE


def run_spmd(pred_points, target_points, sym_flag, trace=False):
    from concourse.bass_utils import run_bass_kernel_spmd

    res = run_bass_kernel_spmd(
        _get_nc(),
        make_in_maps(pred_points, target_points, sym_flag),
        list(range(N_CORES)),
        trace=trace,
    )
    partials = [float(res.results[c]["out"][0, 0]) for c in range(N_CORES)]
    return np.float32(sum(partials) / B), res


def kernel(pred_points, target_points, sym_flag):
    out, _ = run_spmd(pred_points, target_points, sym_flag, trace=False)
    return np.asarray(out, dtype=np.float32)


# revision 22
# speedup vs baseline: 1.5661x; 1.5661x over previous
"""Trainium2 Bass kernel for nn_Loss_17695265260053 (retrieval_knn).

Computes, for B=16 batches of N=2048 3-D points:
  sym[b]  = mean_n min_m ||pred[b,n] - targ[b,m]||      (Chamfer / ADD-S)
  asym[b] = mean_n ||pred[b,n] - targ[b,n]||            (ADD)
  loss    = mean_b (flag[b]*sym[b] + (1-flag[b])*asym[b])

Sharding: data-parallel over batch, 2 batches per core on 8 cores; each
core emits one partial sum, the host sums partials and divides by B.

Per-core algorithm (per batch):
  d2'(n,m) = |t_m|^2 + (-2 p_n).t_m     (|p_n|^2 is added after the min)
  fp32 operands are split error-free into fp16 hi+lo halves and the
  significant products are contracted in a SINGLE K=11 fp16 matmul:
    lhsT = [ph; ph; pl; 1; 1]   (p~ = -2*pred, host-side split, transposed)
    rhs  = [th; tl; th; t2h; t2l] (t transposed hi/lo from host; |t|^2 rows
           computed on device: ScalarE squares + SWDGE accumulate-DMAs for
           the 3-row sum and the fp16 hi/lo residual)
  The dropped lo*lo terms are ~1e-7 relative; fp16 streams at 4x the fp32
  matmul rate on the PE. Per 128-row pred tile the 2048 d2' values land in
  one [128,2048] PSUM tile (4 banks, double-buffered); VectorE min-reduces
  it in a single fused tensor_scalar(op1=min, accum_out) pass.
  Epilogue: + |p|^2, clamp EPS, Sqrt, sum-reduce; a ones-matmul reduces
  across partitions; the sym_flag blend happens on [1,x] lanes.
"""

import sys

for _p in ("/opt/trn_rl_repo", "/opt/pypackages"):
    if _p not in sys.path:
        sys.path.insert(0, _p)

import numpy as np

import concourse.bass as bass
import concourse.tile as tile
from concourse import bacc, mybir

N_CORES = 8
B, N, D = 16, 2048, 3
BPC = B // N_CORES          # batches per core
NT = N // 128               # 16 pred tiles of 128 points
NW = 2048                   # full-width PSUM tile per pred tile
KK = 11                     # contraction: 3 hi*hi + 3 hi*lo + 3 lo*hi + 2 t2
F32 = mybir.dt.float32
F16 = mybir.dt.float16
EPS = 1e-12
Alu = mybir.AluOpType
Act = mybir.ActivationFunctionType


def build_loss_body(nc, tc, predt_d, targt_d, targ32_d, prednat_d, targnat_d,
                    flag_d, out_d):
    """Emit the per-core program.
    predt_d:   [BPC, 11, N] f16 - rows [ph; ph; pl; 1; 1], p~ = -2*pred, transposed
    targt_d:   [BPC, 11, N] f16 - rows [th; tl; th; 0; 0] (t transposed hi/lo)
    targ32_d:  [BPC, 3, N] f32  - t transposed (for |t|^2)
    prednat_d: [BPC, 128, 48] f32 - tiled natural pred ([q, 3t+d] = pt 128t+q)
    targnat_d: [BPC, 128, 48] f32 - tiled natural target
    flag_d: [1, BPC]; out_d: [1, 1]."""
    with (
        tc.tile_pool(name="io", bufs=2) as io,
        tc.tile_pool(name="pre", bufs=2) as pre,
        tc.tile_pool(name="rhs", bufs=2) as rhsp,
        tc.tile_pool(name="acc", bufs=1) as accp,
        tc.tile_pool(name="psum", bufs=2, space="PSUM") as psum,
    ):
        # per-core accumulators / constants
        SSUM = accp.tile([128, 2 * BPC], F32)   # cols: sym0, asym0, sym1, asym1
        ONES = accp.tile([128, 1], F32)
        nc.vector.memset(ONES[:], 1.0 / N)      # folds the 1/N mean into the reduce
        FL = accp.tile([1, BPC], F32)
        nc.sync.dma_start(FL[:], flag_d[:])
        TRA = accp.tile([128, NW], F32)         # reduce elementwise dump

        for b in range(BPC):
            # ---- loads ------------------------------------------------
            P4 = io.tile([128, NT * 3], F32, tag="P4")
            nc.sync.dma_start(P4[:], prednat_d[b])
            T4 = io.tile([128, NT * 3], F32, tag="T4")
            nc.sync.dma_start(T4[:], targnat_d[b])
            LT = rhsp.tile([KK, N], F16, tag="LT")
            nc.sync.dma_start(LT[:], predt_d[b])
            RT = rhsp.tile([KK, N], F16, tag="RT")
            nc.sync.dma_start(RT[:], targt_d[b])
            T3 = rhsp.tile([3, N], F32, tag="T3")
            nc.sync.dma_start(T3[:], targ32_d[b])

            # ---- t2 rows: |t|^2 in row form, exact fp16 hi/lo -----------
            SQ3 = rhsp.tile([3, N], F32, tag="SQ3")
            nc.scalar.activation(SQ3[:], T3[:], Act.Square)
            T2R = rhsp.tile([1, N], F32, tag="T2R")
            nc.gpsimd.dma_start(T2R[:], SQ3[0:1, :])
            nc.gpsimd.dma_start(T2R[:], SQ3[1:2, :], accum_op=Alu.add)
            nc.gpsimd.dma_start(T2R[:], SQ3[2:3, :], accum_op=Alu.add)
            # t2h = fp16(t2); engines need quadrant-aligned partitions, so
            # build the rows at base 0 and DMA them into RT rows 9/10
            T2H16 = rhsp.tile([1, N], F16, tag="T2H16")
            nc.scalar.copy(T2H16[:], T2R[:])
            nc.sync.dma_start(RT[9:10, :], T2H16[:])
            # residual: t2 - fp32(t2h), via negated upcast + accumulate-DMA
            T2HN = rhsp.tile([1, N], F32, tag="T2HN")
            nc.scalar.activation(T2HN[:], T2H16[:], Act.Copy, scale=-1.0)
            T2LR = rhsp.tile([1, N], F32, tag="T2LR")
            nc.gpsimd.dma_start(T2LR[:], T2R[:])
            nc.gpsimd.dma_start(T2LR[:], T2HN[:], accum_op=Alu.add)
            T2L16 = rhsp.tile([1, N], F16, tag="T2L16")
            nc.scalar.copy(T2L16[:], T2LR[:])
            nc.sync.dma_start(RT[10:11, :], T2L16[:])

            # ---- pred prep: |p|^2 and the asym (ADD) branch ------------
            P4SQ = pre.tile([128, NT * 3], F32, tag="p4sq")
            nc.scalar.activation(P4SQ[:], P4[:], Act.Square)
            pv2 = P4SQ.rearrange("q (t d) -> q t d", d=3)
            p2t = pre.tile([128, NT], F32, tag="p2t")
            nc.vector.tensor_add(p2t[:], pv2[:, :, 0], pv2[:, :, 1])
            nc.vector.tensor_add(p2t[:], p2t[:], pv2[:, :, 2])

            ADIF = pre.tile([128, NT * 3], F32, tag="adif")
            nc.vector.tensor_sub(ADIF[:], P4[:], T4[:])
            ASQ = pre.tile([128, NT * 3], F32, tag="asq")
            nc.scalar.activation(ASQ[:], ADIF[:], Act.Square)
            av = ASQ.rearrange("q (t d) -> q t d", d=3)
            AD2 = pre.tile([128, NT], F32, tag="ad2")
            nc.vector.tensor_add(AD2[:], av[:, :, 0], av[:, :, 1])
            nc.vector.tensor_add(AD2[:], AD2[:], av[:, :, 2])
            ASQR = pre.tile([128, NT], F32, tag="asqr")
            nc.scalar.activation(ASQR[:], AD2[:], Act.Sqrt)
            nc.vector.reduce_sum(
                SSUM[:, 2 * b + 1 : 2 * b + 2], ASQR[:], axis=mybir.AxisListType.X
            )

            # ---- main loop: K=11 fp16 matmuls + fused min-reduce -------
            MINS = pre.tile([128, NT], F32, tag="mins")
            for a in range(NT):
                lhs = LT[:, 128 * a : 128 * (a + 1)]
                ps = psum.tile([128, NW], F32, tag="ps")
                if a == 0:
                    # 1-col "toucher" ladder: spread the batch-boundary waits
                    # (psum WAR/WAW, LT DMA, RT DMA + ACT t2 rows) over cheap
                    # matmuls so no LDWEIGHTS exceeds its sync-wait budget.
                    nc.tensor.matmul(
                        ps[0:1, 0:1], ONES[:], ONES[:], start=True, stop=True
                    )
                    nc.tensor.matmul(
                        ps[0:1, 1:2], LT[:, 0:1], LT[:, 0:1], start=True, stop=True
                    )
                    nc.tensor.matmul(
                        ps[0:1, 2:3], RT[:, 0:1], RT[:, 0:1], start=True, stop=True
                    )
                for c in range(4):
                    nc.tensor.matmul(
                        ps[:, 512 * c : 512 * (c + 1)],
                        lhs,
                        RT[:, 512 * c : 512 * (c + 1)],
                        start=True,
                        stop=True,
                    )
                nc.vector.tensor_scalar(
                    TRA[:], ps[:], 0.0, None,
                    op0=Alu.add, op1=Alu.min, accum_out=MINS[:, a : a + 1],
                )

            # ---- epilogue: + |p|^2, clamp, sqrt ------------------------
            D2M = pre.tile([128, NT], F32, tag="d2m")
            nc.vector.tensor_add(D2M[:], p2t[:], MINS[:])
            nc.vector.tensor_scalar_max(D2M[:], D2M[:], EPS)
            DSQ = pre.tile([128, NT], F32, tag="dsq")
            nc.scalar.activation(DSQ[:], D2M[:], Act.Sqrt)
            nc.vector.reduce_sum(
                SSUM[:, 2 * b : 2 * b + 1], DSQ[:], axis=mybir.AxisListType.X
            )

        # ---- final: partition reduce + flag blend ----------------------
        FPS = psum.tile([1, 2 * BPC], F32, tag="ps")
        nc.tensor.matmul(FPS[:], ONES[:], SSUM[:], start=True, stop=True)
        FSB = accp.tile([1, 2 * BPC], F32)
        nc.vector.tensor_copy(FSB[:], FPS[:])
        fv = FSB.rearrange("p (b k) -> p b k", k=2)  # k: 0 = sym, 1 = asym
        T0 = accp.tile([1, BPC], F32)
        nc.vector.tensor_sub(T0[:], fv[:, :, 0], fv[:, :, 1])
        nc.vector.tensor_mul(T0[:], T0[:], FL[:])
        nc.vector.tensor_add(T0[:], T0[:], fv[:, :, 1])
        OUT = accp.tile([1, 1], F32)
        nc.vector.reduce_sum(OUT[:], T0[:], axis=mybir.AxisListType.X)
        nc.sync.dma_start(out_d[:], OUT[:])


def build_core_program():
    """Build the single-core Bass program (same program runs SPMD on all 8)."""
    nc = bacc.Bacc("TRN2", target_bir_lowering=False, debug=False)
    predt_d = nc.dram_tensor("predt", [BPC, KK, N], F16, kind="ExternalInput")
    targt_d = nc.dram_tensor("targt", [BPC, KK, N], F16, kind="ExternalInput")
    targ32_d = nc.dram_tensor("targ32", [BPC, 3, N], F32, kind="ExternalInput")
    prednat_d = nc.dram_tensor("prednat", [BPC, 128, NT * 3], F32, kind="ExternalInput")
    targnat_d = nc.dram_tensor("targnat", [BPC, 128, NT * 3], F32, kind="ExternalInput")
    flag_d = nc.dram_tensor("flag", [1, BPC], F32, kind="ExternalInput")
    out_d = nc.dram_tensor("out", [1, 1], F32, kind="ExternalOutput")
    with tile.TileContext(nc) as tc:
        build_loss_body(nc, tc, predt_d.ap(), targt_d.ap(), targ32_d.ap(),
                        prednat_d.ap(), targnat_d.ap(), flag_d.ap(), out_d.ap())
    nc.compile()
    return nc


def host_inputs(pred_points, targ_points, sym_flag):
    """Host-side input formatting (shard + layout/precision split only)."""
    pred = np.asarray(pred_points, dtype=np.float32)
    targ = np.asarray(targ_points, dtype=np.float32)
    pt = (-2.0 * pred).transpose(0, 2, 1)             # [B, 3, N], exact scaling
    ph = pt.astype(np.float16)
    pl = (pt - ph.astype(np.float32)).astype(np.float16)
    ones = np.ones((B, 1, N), np.float16)
    predt = np.concatenate([ph, ph, pl, ones, ones], axis=1)       # [B, 11, N]
    tt = targ.transpose(0, 2, 1)                      # [B, 3, N]
    th = tt.astype(np.float16)
    tl = (tt - th.astype(np.float32)).astype(np.float16)
    zz = np.zeros((B, 2, N), np.float16)
    targt = np.concatenate([th, tl, th, zz], axis=1)               # [B, 11, N]
    tiled = lambda x: np.ascontiguousarray(
        x.reshape(B, NT, 128, 3).transpose(0, 2, 1, 3).reshape(B, 128, NT * 3)
    )
    return (predt, targt, np.ascontiguousarray(tt), tiled(pred), tiled(targ),
            np.asarray(sym_flag, dtype=np.float32))


def make_in_maps(pred_points, targ_points, sym_flag):
    predt, targt, tt, prednat, targnat, flags = host_inputs(
        pred_points, targ_points, sym_flag
    )
    in_maps = []
    for c in range(N_CORES):
        sl = slice(c * BPC, (c + 1) * BPC)
        in_maps.append(
            {
                "predt": np.ascontiguousarray(predt[sl]),
                "targt": np.ascontiguousarray(targt[sl]),
                "targ32": np.ascontiguousarray(tt[sl]),
                "prednat": np.ascontiguousarray(prednat[sl]),
                "targnat": np.ascontiguousarray(targnat[sl]),
                "flag": np.ascontiguousarray(flags[sl].reshape(1, BPC)),
            }
        )
    return in_maps


_NC_CACHE = None


def _get_nc():
    global _NC_CACHE
    if _NC_CACHE is None:
        _NC_CACHE = build_core_program()
    return _NC_CACHE


def run_spmd(pred_points, target_points, sym_flag, trace=False):
    from concourse.bass_utils import run_bass_kernel_spmd

    res = run_bass_kernel_spmd(
        _get_nc(),
        make_in_maps(pred_points, target_points, sym_flag),
        list(range(N_CORES)),
        trace=trace,
    )
    partials = [float(res.results[c]["out"][0, 0]) for c in range(N_CORES)]
    return np.float32(sum(partials) / B), res


def kernel(pred_points, target_points, sym_flag):
    out, _ = run_spmd(pred_points, target_points, sym_flag, trace=False)
    return np.asarray(out, dtype=np.float32)
